# revision 1
# baseline (speedup 1.0000x reference)
"""Trainium2 Bass kernel for CrossModalAttn.

reference:
    q = conv1x1(c, Wq, bq); k = conv1x1(p, Wk, bk); v = conv1x1(p, Wv, bv)
    y = softmax(q*k, axis=-1) * v          # softmax over width W
    y = conv1x1(y, Wy, by)
    out = concat(y, c, axis=1)             # [B, 64, H, W]

Sharding: pure data parallel, B=16 batches over 8 cores (2 per core),
weights replicated.

Per-core layout: each batch plane [32ch, 65536] is split into 4 spatial
quarters stacked on the 128 SBUF partitions (quarter-major, 32 channels
each).  The 1x1 convs are block-diagonal matmuls (K=128) so one matmul
instruction services all four quarters.  Softmax width-segments (256)
lie along the free dimension; exp+segment-sum is fused on ScalarE via
accum_out, normalization is a per-segment tensor_scalar broadcast.
"""

import sys
from contextlib import ExitStack

import numpy as np

sys.path.insert(0, "/opt/trn_rl_repo")

import concourse.bacc as bacc
import concourse.bass as bass  # noqa: F401
import concourse.mybir as mybir
import concourse.tile as tile
from concourse import bass_utils

F32 = mybir.dt.float32
AF = mybir.ActivationFunctionType

B, CIN, H, W = 16, 32, 256, 256
CY = 16
NCORES = 8
BLOC = B // NCORES          # batches per core
HW = H * W                  # 65536
QN = 4                      # spatial quarters stacked on partitions
QSEG = HW // QN             # 16384 positions per quarter
SEG = W                     # softmax segment length (one h-row)

FREE = 4096                 # free columns per SBUF io tile (per quarter)
CH = 1024                   # free columns per compute chunk (PSUM resident)

_CACHE: dict = {}


def _block_diag(blk: np.ndarray, n: int) -> np.ndarray:
    """[K, M] block -> [n*K, n*M] block-diagonal."""
    k, m = blk.shape
    out = np.zeros((n * k, n * m), np.float32)
    for i in range(n):
        out[i * k:(i + 1) * k, i * m:(i + 1) * m] = blk
    return out


def build_module():
    nc = bacc.Bacc("TRN2", target_bir_lowering=False, debug=False)

    cD = nc.dram_tensor("c", [BLOC, CIN, HW], F32, kind="ExternalInput").ap()
    pD = nc.dram_tensor("p", [BLOC, CIN, HW], F32, kind="ExternalInput").ap()
    wqD = nc.dram_tensor("wq", [128, QN * CY], F32, kind="ExternalInput").ap()
    wkD = nc.dram_tensor("wk", [128, QN * CY], F32, kind="ExternalInput").ap()
    wvD = nc.dram_tensor("wv", [128, QN * CY], F32, kind="ExternalInput").ap()
    wyD = nc.dram_tensor("wy", [QN * CY, 128], F32, kind="ExternalInput").ap()
    bqD = nc.dram_tensor("bq", [QN * CY, 1], F32, kind="ExternalInput").ap()
    bkD = nc.dram_tensor("bk", [QN * CY, 1], F32, kind="ExternalInput").ap()
    bvD = nc.dram_tensor("bv", [QN * CY, 1], F32, kind="ExternalInput").ap()
    byD = nc.dram_tensor("by", [128, 1], F32, kind="ExternalInput").ap()
    oD = nc.dram_tensor("out", [BLOC, 2 * CIN, HW], F32, kind="ExternalOutput").ap()

    with tile.TileContext(nc) as tc, ExitStack() as ctx:
        const = ctx.enter_context(tc.tile_pool(name="const", bufs=1))
        io = ctx.enter_context(tc.tile_pool(name="io", bufs=2))
        work = ctx.enter_context(tc.tile_pool(name="work", bufs=2))
        small = ctx.enter_context(tc.tile_pool(name="small", bufs=4))
        psq = ctx.enter_context(tc.tile_pool(name="psq", bufs=1, space="PSUM"))
        psk = ctx.enter_context(tc.tile_pool(name="psk", bufs=1, space="PSUM"))
        psv = ctx.enter_context(tc.tile_pool(name="psv", bufs=1, space="PSUM"))
        psy = ctx.enter_context(tc.tile_pool(name="psy", bufs=1, space="PSUM"))

        wq = const.tile([128, QN * CY], F32)
        nc.sync.dma_start(wq[:], wqD)
        wk = const.tile([128, QN * CY], F32)
        nc.sync.dma_start(wk[:], wkD)
        wv = const.tile([128, QN * CY], F32)
        nc.sync.dma_start(wv[:], wvD)
        wy = const.tile([QN * CY, 128], F32)
        nc.sync.dma_start(wy[:], wyD)
        bq_t = const.tile([QN * CY, 1], F32)
        nc.sync.dma_start(bq_t[:], bqD)
        bk_t = const.tile([QN * CY, 1], F32)
        nc.sync.dma_start(bk_t[:], bkD)
        bv_t = const.tile([QN * CY, 1], F32)
        nc.sync.dma_start(bv_t[:], bvD)
        by_t = const.tile([128, 1], F32)
        nc.sync.dma_start(by_t[:], byD)

        for b in range(BLOC):
            for it in range(QSEG // FREE):
                off = it * FREE
                c_sb = io.tile([128, FREE], F32, tag="c_sb")
                p_sb = io.tile([128, FREE], F32, tag="p_sb")
                y_sb = io.tile([128, FREE], F32, tag="y_sb")
                for qt in range(QN):
                    sl = slice(qt * QSEG + off, qt * QSEG + off + FREE)
                    pr = slice(32 * qt, 32 * (qt + 1))
                    nc.sync.dma_start(c_sb[pr, :], cD[b][:, sl])
                    nc.sync.dma_start(p_sb[pr, :], pD[b][:, sl])

                for chk in range(FREE // CH):
                    csl = c_sb[:, chk * CH:(chk + 1) * CH]
                    psl = p_sb[:, chk * CH:(chk + 1) * CH]
                    q0 = psq.tile([QN * CY, CH], F32, tag="q0")
                    k0 = psk.tile([QN * CY, CH], F32, tag="k0")
                    v0 = psv.tile([QN * CY, CH], F32, tag="v0")
                    for mm in range(CH // 512):
                        s2 = slice(mm * 512, (mm + 1) * 512)
                        nc.tensor.matmul(q0[:, s2], wq[:], csl[:, s2],
                                         start=True, stop=True)
                        nc.tensor.matmul(k0[:, s2], wk[:], psl[:, s2],
                                         start=True, stop=True)
                        nc.tensor.matmul(v0[:, s2], wv[:], psl[:, s2],
                                         start=True, stop=True)

                    q_sb = work.tile([QN * CY, CH], F32, tag="q_sb")
                    nc.scalar.activation(q_sb[:], q0[:], AF.Identity, bias=bq_t[:])
                    k_sb = work.tile([QN * CY, CH], F32, tag="k_sb")
                    nc.scalar.activation(k_sb[:], k0[:], AF.Identity, bias=bk_t[:])
                    v_sb = work.tile([QN * CY, CH], F32, tag="v_sb")
                    nc.vector.tensor_scalar_add(v_sb[:], v0[:], bv_t[:])

                    s_sb = work.tile([QN * CY, CH], F32, tag="s_sb")
                    nc.vector.tensor_mul(s_sb[:], q_sb[:], k_sb[:])

                    e_sb = work.tile([QN * CY, CH], F32, tag="e_sb")
                    sums = small.tile([QN * CY, CH // SEG], F32, tag="sums")
                    for g in range(CH // SEG):
                        gs = slice(g * SEG, (g + 1) * SEG)
                        nc.scalar.activation(e_sb[:, gs], s_sb[:, gs], AF.Exp,
                                             accum_out=sums[:, g:g + 1])
                    r = small.tile([QN * CY, CH // SEG], F32, tag="r")
                    nc.vector.reciprocal(r[:], sums[:])

                    t_sb = work.tile([QN * CY, CH], F32, tag="t_sb")
                    nc.vector.tensor_mul(t_sb[:], e_sb[:], v_sb[:])
                    yn = work.tile([QN * CY, CH], F32, tag="yn")
                    for g in range(CH // SEG):
                        gs = slice(g * SEG, (g + 1) * SEG)
                        nc.vector.tensor_scalar_mul(yn[:, gs], t_sb[:, gs],
                                                    r[:, g:g + 1])

                    y0 = psy.tile([128, CH], F32, tag="y0")
                    for mm in range(CH // 512):
                        s2 = slice(mm * 512, (mm + 1) * 512)
                        nc.tensor.matmul(y0[:, s2], wy[:], yn[:, s2],
                                         start=True, stop=True)
                    nc.scalar.activation(y_sb[:, chk * CH:(chk + 1) * CH], y0[:],
                                         AF.Identity, bias=by_t[:])

                for qt in range(QN):
                    sl = slice(qt * QSEG + off, qt * QSEG + off + FREE)
                    pr = slice(32 * qt, 32 * (qt + 1))
                    nc.sync.dma_start(oD[b][0:CIN, sl], y_sb[pr, :])
                    nc.sync.dma_start(oD[b][CIN:2 * CIN, sl], c_sb[pr, :])

    nc.compile()
    return nc


def _prep_consts(Wq, bq, Wk, bk, Wv, bv, Wy, by):
    f = np.float32
    return {
        "wq": _block_diag(np.ascontiguousarray(np.asarray(Wq, f).T), QN),
        "wk": _block_diag(np.ascontiguousarray(np.asarray(Wk, f).T), QN),
        "wv": _block_diag(np.ascontiguousarray(np.asarray(Wv, f).T), QN),
        "wy": _block_diag(np.ascontiguousarray(np.asarray(Wy, f).T), QN),
        "bq": np.tile(np.asarray(bq, f), QN)[:, None].copy(),
        "bk": np.tile(np.asarray(bk, f), QN)[:, None].copy(),
        "bv": np.tile(np.asarray(bv, f), QN)[:, None].copy(),
        "by": np.tile(np.asarray(by, f), QN)[:, None].copy(),
    }


LAST_RESULT = None


def kernel(c, p, Wq, bq, Wk, bk, Wv, bv, Wy, by):
    global LAST_RESULT
    nc = _CACHE.get("nc")
    if nc is None:
        nc = _CACHE["nc"] = build_module()

    consts = _prep_consts(Wq, bq, Wk, bk, Wv, bv, Wy, by)
    c = np.asarray(c, np.float32).reshape(B, CIN, HW)
    p = np.asarray(p, np.float32).reshape(B, CIN, HW)

    in_maps = []
    for m in range(NCORES):
        sl = slice(m * BLOC, (m + 1) * BLOC)
        in_maps.append({
            "c": np.ascontiguousarray(c[sl]),
            "p": np.ascontiguousarray(p[sl]),
            **consts,
        })

    res = bass_utils.run_bass_kernel_spmd(nc, in_maps, core_ids=list(range(NCORES)))
    LAST_RESULT = res
    out = np.concatenate([res.results[i]["out"] for i in range(NCORES)], axis=0)
    return out.reshape(B, 2 * CIN, H, W)
